# revision 9
# baseline (speedup 1.0000x reference)
"""Trainium2 Bass kernel for nn_Attention_37847251812733.

Full transformer block: QKV proj -> 16-head attention (N=4096, DH=64)
-> permuted reshape (the reference's transpose(1,2).reshape) -> LN ->
MLP -> LN.

Sharding: tensor-parallel over heads; core c owns heads {2c, 2c+1}.
The reference's att reshape maps head h's transposed attention output
att.T[d, n] to rows r = (h*64+d)*4 + n_hi of the permuted [4096, 1024]
tensor, so core c produces exactly rows [512c, 512c+512) of everything
downstream (LN1/MLP/LN2 are row-wise) -> no collectives needed.

Per-core dataflow:
  P1: QT/KT (bf16 [128, 4096], dh-pair on partitions) + V via f32r
      matmuls from host-pretransposed xT; V transposed to [kpos, dh]
      blocks with a TRAILING ones column ([V|1]) so the attention
      matmul also emits softmax denominators (at out partition 64).
  P2: per q-tile(512): scoresT [kpos, q] via 2-head row-packed K=64
      bf16 matmuls into a 6-bank PSUM batch (3 kb-pairs); one Exp ACT
      call per batch (scale=1/8 folded, no max subtraction -- scores
      bounded ~|2|); att.T accumulated from [1|V] stationary matmuls
      into 2 PSUM banks (two 16/17-block chunks, drained to SBUF);
      normalize via reciprocal + K=1 ones-matmul partition broadcast;
      DMA-scatter into the permuted row layout.
  P3: LN1 (bn_stats/aggr) -> PE-transpose h1 -> f32r MLP (+b1,
      residual) -> LN2 -> out rows.
"""
import sys

if "/opt/trn_rl_repo" not in sys.path:
    sys.path.insert(0, "/opt/trn_rl_repo")

import numpy as np
from contextlib import ExitStack

import concourse.bacc as bacc
import concourse.mybir as mybir
import concourse.tile as tile
from concourse import bass2jax

f32 = mybir.dt.float32
f32r = mybir.dt.float32r
bf16 = mybir.dt.bfloat16
Exp = mybir.ActivationFunctionType.Exp
Sqrt = mybir.ActivationFunctionType.Sqrt
Alu = mybir.AluOpType

N, D = 4096, 1024
EPS = 1e-5
ROWS = 512            # rows of the permuted tensor per core
NT = 8                # 512-wide tiles
KB = 32               # kpos blocks of 128 per q-tile


def build(loop=0, phases=3, timing_reps=0, internal=False):
    """Build the per-core SPMD program. loop>0 wraps the body in For_i
    (timing variant)."""
    nc = bacc.Bacc("TRN2", target_bir_lowering=False, debug=False, num_devices=8)

    kind = "Internal" if (timing_reps or internal) else "ExternalInput"
    xT_d = nc.dram_tensor("xT", [D, N], f32r, kind=kind).ap()
    wqT_d = nc.dram_tensor("wqT", [D, 128], f32r, kind=kind).ap()
    wkT_d = nc.dram_tensor("wkT", [D, 128], f32r, kind=kind).ap()
    wvT_d = nc.dram_tensor("wvT", [D, 128], f32r, kind=kind).ap()
    w1T_d = nc.dram_tensor("w1T", [D, D], f32r, kind=kind).ap()
    xres_d = nc.dram_tensor("xres", [ROWS, D], f32, kind=kind).ap()
    b1b_d = nc.dram_tensor("b1b", [128, D], f32, kind=kind).ap()
    g1b_d = nc.dram_tensor("g1b", [128, D], f32, kind=kind).ap()
    bb1_d = nc.dram_tensor("bb1", [128, D], f32, kind=kind).ap()
    g2b_d = nc.dram_tensor("g2b", [128, D], f32, kind=kind).ap()
    bb2_d = nc.dram_tensor("bb2", [128, D], f32, kind=kind).ap()
    ones_d = nc.dram_tensor("ones64", [1, 64], f32r, kind=kind).ap()
    idbf_d = nc.dram_tensor("idbf", [128, 128], bf16, kind=kind).ap()
    idf_d = nc.dram_tensor("idf", [128, 128], f32, kind=kind).ap()
    if timing_reps or internal:
        out_d = nc.dram_tensor("out", [ROWS, D], f32, kind="Internal").ap()
        tick_d = nc.dram_tensor("tick", [1, 4], f32, kind="ExternalOutput").ap()
    else:
        out_d = nc.dram_tensor("out", [ROWS, D], f32, kind="ExternalOutput").ap()
        tick_d = None

    with tile.TileContext(nc) as tc:
        with ExitStack() as ctx:
            const = ctx.enter_context(tc.tile_pool(name="const", bufs=1))
            main = ctx.enter_context(tc.tile_pool(name="main", bufs=1))

            # constants loaded once (outside any timing loop)
            w_tiles = {}
            for pname, dram in (("wq", wqT_d), ("wk", wkT_d), ("wv", wvT_d)):
                for k in range(8):
                    t = const.tile([128, 128], f32r, name=f"{pname}T{k}")
                    nc.sync.dma_start(t[:], dram[128 * k:128 * (k + 1), :])
                    w_tiles[(pname, k)] = t
            w1T_t = []
            for k in range(8):
                t = const.tile([128, D], f32r, name=f"w1T{k}")
                nc.sync.dma_start(t[:], w1T_d[128 * k:128 * (k + 1), :])
                w1T_t.append(t)
            prm = {}
            for pname, dram in (("b1b", b1b_d), ("g1b", g1b_d), ("bb1", bb1_d),
                                ("g2b", g2b_d), ("bb2", bb2_d)):
                t = const.tile([128, D], f32, name=pname)
                nc.sync.dma_start(t[:], dram[:])
                prm[pname] = t
            ones_t = const.tile([1, 64], f32r)
            nc.sync.dma_start(ones_t[:], ones_d[:])
            idbf_t = const.tile([128, 128], bf16)
            nc.sync.dma_start(idbf_t[:], idbf_d[:])
            idf_t = const.tile([128, 128], f32)
            nc.sync.dma_start(idf_t[:], idf_d[:])
            eps_t = const.tile([128, 1], f32)
            nc.vector.memset(eps_t[:], EPS)
            xres_t = []
            for r in range(4):
                t = const.tile([128, D], f32, name=f"xres{r}")
                nc.sync.dma_start(t[:], xres_d[128 * r:128 * (r + 1), :])
                xres_t.append(t)

            # persistent working tensors
            QT = main.tile([128, N], bf16)        # [dh-pair, n]
            KT = main.tile([128, N], bf16)
            VB = [main.tile([128, 65 * KB], bf16, name=f"VB{h}") for h in range(2)]
            for h in range(2):
                nc.vector.memset(VB[h][:], 1.0)
            att_perm = [main.tile([128, D], f32, name=f"attperm{r}") for r in range(4)]

            def body(_=None):
                # ================= P1: QKV projections ====================
                if phases < 1:
                    nc.vector.memset(att_perm[0][:], 0.0)
                    return
                with ExitStack() as c1:
                    p1sb = c1.enter_context(tc.tile_pool(name="p1sb", bufs=1))
                    p1ps = c1.enter_context(tc.tile_pool(name="p1ps", bufs=2, space="PSUM"))
                    trps = c1.enter_context(tc.tile_pool(name="trps", bufs=2, space="PSUM"))

                    VT = p1sb.tile([128, N], bf16)
                    for nt in range(NT):
                        xc = [p1sb.tile([128, 512], f32r, name=f"xc{k}", tag=f"xc{k}", bufs=2)
                              for k in range(8)]
                        for k in range(8):
                            nc.sync.dma_start(xc[k][:], xT_d[128 * k:128 * (k + 1),
                                                             512 * nt:512 * (nt + 1)])
                        qps = p1ps.tile([128, 512], f32, tag="qps")
                        kps = p1ps.tile([128, 512], f32, tag="kps")
                        vps = p1ps.tile([128, 512], f32, tag="vps")
                        for k in range(8):
                            st = (k == 0)
                            sp = (k == 7)
                            nc.tensor.matmul(qps[:], w_tiles[("wq", k)][:], xc[k][:], start=st, stop=sp)
                            nc.tensor.matmul(kps[:], w_tiles[("wk", k)][:], xc[k][:], start=st, stop=sp)
                            nc.tensor.matmul(vps[:], w_tiles[("wv", k)][:], xc[k][:], start=st, stop=sp)
                        sl = slice(512 * nt, 512 * (nt + 1))
                        nc.vector.tensor_copy(QT[:, sl], qps[:])
                        nc.vector.tensor_copy(KT[:, sl], kps[:])
                        nc.vector.tensor_copy(VT[:, sl], vps[:])
                        for b in range(4):
                            kb = 4 * nt + b
                            tp = trps.tile([128, 128], bf16, tag="tp")
                            nc.tensor.transpose(
                                tp[:], VT[:, 512 * nt + 128 * b:512 * nt + 128 * (b + 1)],
                                idbf_t[:])
                            for h in range(2):
                                nc.vector.tensor_copy(
                                    VB[h][:, 65 * kb:65 * kb + 64],
                                    tp[:, 64 * h:64 * (h + 1)])

                if phases < 2:
                    nc.vector.tensor_copy(att_perm[0][:, 0:512], QT[:, 0:512])
                    return
                # ================= P2: attention ==========================
                with ExitStack() as c2:
                    p2sb = c2.enter_context(tc.tile_pool(name="p2sb", bufs=1))
                    scps = c2.enter_context(tc.tile_pool(name="scps", bufs=1, space="PSUM"))
                    atps = c2.enter_context(tc.tile_pool(name="atps", bufs=1, space="PSUM"))

                    # kb batches of 3 pairs -> 6 psum banks; tail batch of 2
                    batches = [list(range(i, min(i + 3, KB))) for i in range(0, KB, 3)]
                    CH = 4  # att chunk 1 = kb 0..14 (batches 0..4)

                    for qt in range(NT):
                        qsl = slice(512 * qt, 512 * (qt + 1))
                        attps = [atps.tile([65, 512], f32, name=f"attps{h}", tag=f"attps{h}")
                                 for h in range(2)]
                        acc = [p2sb.tile([65, 512], f32, name=f"acc{h}", tag=f"acc{h}", bufs=3)
                               for h in range(2)]
                        pend = None  # software pipeline: att MMs lag one batch
                        for bi, kbs in enumerate(batches):
                            nb = len(kbs)
                            scab = scps.tile([128, 3072], f32, tag="scab")
                            for j, kb in enumerate(kbs):
                                ksl = slice(128 * kb, 128 * (kb + 1))
                                nc.tensor.matmul(scab[:, 1024 * j:1024 * j + 512],
                                                 KT[0:64, ksl], QT[0:64, qsl],
                                                 start=True, stop=True)
                                nc.tensor.matmul(scab[:, 1024 * j + 512:1024 * (j + 1)],
                                                 KT[64:128, ksl], QT[64:128, qsl],
                                                 start=True, stop=True)
                            eab = p2sb.tile([128, 3072], bf16, tag="eab", bufs=4)
                            nc.scalar.activation(eab[:, 0:1024 * nb], scab[:, 0:1024 * nb],
                                                 Exp, scale=0.125)

                            def att_mms(kbs_, eab_):
                                for j_, kb_ in enumerate(kbs_):
                                    st = kb_ in (0, 15)
                                    sp = kb_ in (14, 31)
                                    for h in range(2):
                                        nc.tensor.matmul(
                                            attps[h][:],
                                            VB[h][:, 65 * kb_:65 * (kb_ + 1)],
                                            eab_[:, 1024 * j_ + 512 * h:1024 * j_ + 512 * (h + 1)],
                                            start=st, stop=sp)

                            if pend is not None:
                                att_mms(*pend)
                            pend = (kbs, eab)
                            if bi == CH + 1:
                                # chunk 1 (kb 0..14) fully accumulated after
                                # batch CH's att MMs ran (they lag by one)
                                for h in range(2):
                                    nc.vector.tensor_copy(acc[h][:], attps[h][:])
                        att_mms(*pend)

                        # normalize + scatter
                        n_hi = qt // 2
                        c0 = 512 * (qt % 2)
                        for h in range(2):
                            nc.vector.tensor_add(acc[h][:], acc[h][:], attps[h][:])
                            rsum = p2sb.tile([1, 512], f32, tag="rsum", bufs=2)
                            nc.vector.tensor_copy(rsum[:], acc[h][64:65, :])
                            rec = p2sb.tile([1, 512], f32, tag="rec", bufs=2)
                            nc.vector.reciprocal_approx_fast(rec[:], rsum[:])
                            rec_r = p2sb.tile([1, 512], f32r, tag="recr", bufs=2)
                            nc.vector.tensor_copy(rec_r[:], rec[:].bitcast(f32r))
                            # K=1 partition-broadcast matmul into the (now
                            # drained) att psum bank
                            nc.tensor.matmul(attps[h][0:64, :], ones_t[:], rec_r[:],
                                             start=True, stop=True)
                            anorm = p2sb.tile([64, 512], f32, tag="anorm", bufs=3)
                            nc.vector.tensor_mul(anorm[:], acc[h][0:64, :], attps[h][0:64, :])
                            nc.sync.dma_start(
                                att_perm[2 * h][n_hi:128:4, c0:c0 + 512], anorm[0:32, :])
                            nc.sync.dma_start(
                                att_perm[2 * h + 1][n_hi:128:4, c0:c0 + 512], anorm[32:64, :])

                if phases < 3:
                    return
                # ================= P3: LN1 -> MLP -> LN2 ==================
                with ExitStack() as c3:
                    p3sb = c3.enter_context(tc.tile_pool(name="p3sb", bufs=1))
                    mlps = c3.enter_context(tc.tile_pool(name="mlps", bufs=2, space="PSUM"))
                    trp3 = c3.enter_context(tc.tile_pool(name="trp3", bufs=2, space="PSUM"))

                    def layer_norm(dst, src, g_t, b_t, tag):
                        st6 = p3sb.tile([128, 2, 6], f32, name=f"st6{tag}", tag="st6", bufs=2)
                        nc.vector.bn_stats(st6[:, 0, :], src[:, 0:512])
                        nc.vector.bn_stats(st6[:, 1, :], src[:, 512:1024])
                        ag = p3sb.tile([128, 2], f32, name=f"ag{tag}", tag="ag", bufs=2)
                        nc.vector.bn_aggr(ag[:], st6[:])
                        nmu = p3sb.tile([128, 1], f32, name=f"nmu{tag}", tag="nmu", bufs=2)
                        nc.vector.tensor_scalar_mul(nmu[:], ag[:, 0:1], -1.0)
                        sd = p3sb.tile([128, 1], f32, name=f"sd{tag}", tag="sd", bufs=2)
                        nc.scalar.activation(sd[:], ag[:, 1:2], Sqrt, bias=eps_t[:])
                        inv = p3sb.tile([128, 1], f32, name=f"inv{tag}", tag="inv", bufs=2)
                        nc.vector.reciprocal(inv[:], sd[:])
                        nc.vector.tensor_scalar(dst[:], src[:], nmu[:], inv[:],
                                                op0=Alu.add, op1=Alu.mult)
                        nc.vector.tensor_mul(dst[:], dst[:], g_t[:])
                        nc.vector.tensor_add(dst[:], dst[:], b_t[:])

                    h1 = [p3sb.tile([128, D], f32, name=f"h1_{r}") for r in range(4)]
                    h1T = [p3sb.tile([128, 512], f32r, name=f"h1T{c}") for c in range(8)]
                    for r in range(4):
                        s1 = p3sb.tile([128, D], f32, tag="s1", bufs=2)
                        nc.vector.tensor_add(s1[:], att_perm[r][:], xres_t[r][:])
                        layer_norm(h1[r], s1, prm["g1b"], prm["bb1"], f"a{r}")
                        for cb in range(8):
                            tp = trp3.tile([128, 128], f32, tag="tpf")
                            nc.tensor.transpose(tp[:], h1[r][:, 128 * cb:128 * (cb + 1)], idf_t[:])
                            nc.vector.tensor_copy(h1T[cb][:, 128 * r:128 * (r + 1)],
                                                  tp[:].bitcast(f32r))
                        s2 = p3sb.tile([128, D], f32, tag="s2", bufs=2)
                        for jt in range(2):
                            jsl = slice(512 * jt, 512 * (jt + 1))
                            mp = mlps.tile([128, 512], f32, tag="mp")
                            for cb in range(8):
                                nc.tensor.matmul(mp[:], h1T[cb][:, 128 * r:128 * (r + 1)],
                                                 w1T_t[cb][:, jsl],
                                                 start=(cb == 0), stop=(cb == 7))
                            nc.vector.scalar_tensor_tensor(
                                s2[:, jsl], mp[:], 1.0, prm["b1b"][:, jsl],
                                op0=Alu.mult, op1=Alu.add)
                            nc.vector.tensor_add(s2[:, jsl], s2[:, jsl], h1[r][:, jsl])
                        o_t = p3sb.tile([128, D], f32, tag="ot", bufs=2)
                        layer_norm(o_t, s2, prm["g2b"], prm["bb2"], f"b{r}")
                        nc.sync.dma_start(out_d[128 * r:128 * (r + 1), :], o_t[:])

            if timing_reps:
                for _rep in range(timing_reps):
                    body()
            elif loop:
                with tc.For_i(0, loop, 1) as _:
                    body()
            else:
                body()
            if tick_d is not None:
                tick = main.tile([1, 4], f32)
                nc.vector.tensor_copy(tick[:], att_perm[0][0:1, 0:4])
                nc.sync.dma_start(tick_d[:], tick[:])
    nc.compile()
    return nc


_CACHE = {}


def _get_nc(loop=0, phases=3, timing_reps=0, internal=False):
    key = (loop, phases, timing_reps, internal)
    if key not in _CACHE:
        _CACHE[key] = build(loop, phases, timing_reps, internal)
    return _CACHE[key]


def make_in_maps(x, wq, wk, wv, ln1_g, ln1_b, w1, b1, ln2_g, ln2_b):
    import ml_dtypes
    x = np.asarray(x, np.float32)
    xT = np.ascontiguousarray(x.T)
    w1T = np.ascontiguousarray(np.asarray(w1, np.float32).T)
    bcast = lambda v: np.ascontiguousarray(
        np.broadcast_to(np.asarray(v, np.float32), (128, D)))
    idf = np.eye(128, dtype=np.float32)
    idbf = np.eye(128, dtype=ml_dtypes.bfloat16)
    ones64 = np.ones((1, 64), np.float32)
    in_maps = []
    for c in range(8):
        rs = slice(128 * c, 128 * (c + 1))
        in_maps.append({
            "xT": xT,
            "wqT": np.ascontiguousarray(np.asarray(wq, np.float32)[rs].T),
            "wkT": np.ascontiguousarray(np.asarray(wk, np.float32)[rs].T),
            "wvT": np.ascontiguousarray(np.asarray(wv, np.float32)[rs].T),
            "w1T": w1T,
            "xres": np.ascontiguousarray(x[512 * c:512 * (c + 1)]),
            "b1b": bcast(b1), "g1b": bcast(ln1_g), "bb1": bcast(ln1_b),
            "g2b": bcast(ln2_g), "bb2": bcast(ln2_b),
            "ones64": ones64, "idbf": idbf, "idf": idf,
        })
    return in_maps


def kernel(**inputs):
    nc = _get_nc(0)
    in_maps = make_in_maps(**inputs)
    results = bass2jax.run_bass_via_pjrt(nc, in_maps, n_cores=8)
    out = np.concatenate([results[c]["out"] for c in range(8)], axis=0)
    return out.astype(np.float32)
